# revision 32
# baseline (speedup 1.0000x reference)
"""Efficient Channel Attention kernel for 8 Trainium2 NeuronCores.

Problem (B=4, N=4096, C=1024, H=4, HD=256):
    qkv = x @ Wqkv.T                 -> q,k,v per head
    q,k l2-normalized over N; scores = (q*temp) @ k.T   [HD, HD] per (b,h)
    attn = softmax(scores, -1); out = attn @ v; y = out @ Wproj.T + bproj + x

Sharding: core = (batch b, token-half). All channel contractions are local;
the only cross-core coupling is the token(N)-contracted quantities: the
Grams k^T q and the q/k squared norms, AllReduce'd within the core pair
sharing a batch. The reduction is split by HEAD-PAIR: q,k for heads 0,1 are
computed first, their Gram+norms AllReduce (#1, 528KB) flies while heads
2,3 are computed (#2 likewise overlaps the v pass), so neither collective's
~25us latency is exposed.

All large GEMMs run in fp8(e4m3) with MatmulPerfMode.DoubleRow: operands
laid out [K=128, 2, free] (two 128-row contraction groups per instruction,
0.5 cycles/row = 2x bf16/f32r PE rate). Weights are pre-scaled by WS=32 on
the host so W entries sit in fp8's normal range (q/k/v reach ~7.7 abs, so
32x keeps everything under fp8 e4m3's 448 max). The power-of-two scales
cancel in l2-normalization and are folded into the norm reciprocals and the
final projection bias-activation (scale=1/WS^2).

PSUM discipline (8 banks): sA-sD [128,512] (1 bank each) + gA,gB
[128,1024] (2 banks each). A1 q/k tiles double-buffer across sA-sD while
gA/gB hold Gram/norm accumulators; in phase B/C, attention h uses
sA/sB/gA/gB while projection h rides sC/sD under attention h+1.
"""

import numpy as np
import ml_dtypes

B, N, C, H = 4, 4096, 1024, 4
HD = C // H          # 256
NCORES = 8
NL = N // 2          # 2048 tokens per core
WS = 32.0            # host-side weight prescale
F8 = ml_dtypes.float8_e4m3
BF16 = ml_dtypes.bfloat16

_CACHE = {}


def _build():
    import concourse.mybir as mybir
    import concourse.tile as tile
    from concourse import bacc
    from concourse.masks import make_identity

    f32 = mybir.dt.float32
    bf = mybir.dt.bfloat16
    f8 = mybir.dt.float8e4
    DR = mybir.MatmulPerfMode.DoubleRow
    AX = mybir.AxisListType.X
    ADD = mybir.AluOpType.add
    Exp = mybir.ActivationFunctionType.Exp
    Ident = mybir.ActivationFunctionType.Identity
    Square = mybir.ActivationFunctionType.Square
    Sqrt = mybir.ActivationFunctionType.Sqrt

    nc = bacc.Bacc("TRN2", target_bir_lowering=False, debug=False,
                   num_devices=NCORES)

    x8_d = nc.dram_tensor("x8", [128, 4, 2, NL], f8, kind="ExternalInput").ap()
    # wqkA = (q cols 0:512 | k cols 0:512) i.e. heads 0,1; wqkB = heads 2,3
    wqkA_d = nc.dram_tensor("wqkA", [128, 4, 2, 1024], f8, kind="ExternalInput").ap()
    wqkB_d = nc.dram_tensor("wqkB", [128, 4, 2, 1024], f8, kind="ExternalInput").ap()
    wv8_d = nc.dram_tensor("wv8", [128, 4, 2, 1024], f8, kind="ExternalInput").ap()
    wp8_d = nc.dram_tensor("wp8", [128, 4, 2, 1024], f8, kind="ExternalInput").ap()
    xr_d = nc.dram_tensor("xr", [128, 8, NL], bf, kind="ExternalInput").ap()
    bias_d = nc.dram_tensor("bias", [128, 8], f32, kind="ExternalInput").ap()
    tmpv_d = nc.dram_tensor("tmpv", [128, 8], f32, kind="ExternalInput").ap()
    yT_d = nc.dram_tensor("yT", [C, NL], bf, kind="ExternalOutput").ap()

    with tile.TileContext(nc) as tc:
        with (
            tc.tile_pool(name="const", bufs=1) as constp,
            tc.tile_pool(name="wgt", bufs=1) as wgtp,
            tc.tile_pool(name="xs", bufs=1) as xsp,
            tc.tile_pool(name="qk", bufs=1) as qkp,
            tc.tile_pool(name="vo", bufs=1) as vop,
            tc.tile_pool(name="wrk", bufs=1) as wrk,
            tc.tile_pool(name="ps1", bufs=1, space="PSUM") as ps1,
            tc.tile_pool(name="dram", bufs=1, space="DRAM") as dramp,
        ):
            # ---------------- constants ----------------
            ident = constp.tile([128, 128], f32, name="ident")
            make_identity(nc, ident[:])
            identb = constp.tile([128, 128], bf, name="identb")
            nc.gpsimd.tensor_copy(identb[:], ident[:])
            bias_sb = constp.tile([128, 8], f32, name="bias_sb")
            nc.sync.dma_start(bias_sb[:], bias_d[:])
            tmpv_sb = constp.tile([128, 8], f32, name="tmpv_sb")
            nc.sync.dma_start(tmpv_sb[:], tmpv_d[:])
            ones8 = constp.tile([128, 2, 128], f8, name="ones8")
            nc.vector.memset(ones8[:], 1.0)

            # ---------------- bulk input DMA (all SBUF-resident) ---------
            # many mid-size DMAs stream on parallel DMA engines,
            # first-needed-first: x token-half 0 + wqkA gate the first
            # matmul (~1.5MB), everything else arrives behind it
            # issue queues round-robin so 600ns/issue doesn't serialize
            xh = [[None, None] for _ in range(4)]
            for t in range(4):
                xt = xsp.tile([128, 2, 1024], f8, tag=f"x{t}0", name=f"x8_{t}0")
                nc.sync.dma_start(xt[:], x8_d[:, t, :, 0:1024])
                xh[t][0] = xt
            wqkA8 = []
            for t in range(4):
                wt = wgtp.tile([128, 2, 1024], f8, tag=f"wqkA{t}",
                               name=f"wqkA8_{t}")
                nc.sync.dma_start(wt[:], wqkA_d[:, t])
                wqkA8.append(wt)
            for t in range(4):
                xt = xsp.tile([128, 2, 1024], f8, tag=f"x{t}1", name=f"x8_{t}1")
                nc.sync.dma_start(xt[:], x8_d[:, t, :, 1024:2048])
                xh[t][1] = xt
            wqkB8, wv8, wp8 = [], [], []
            for t in range(4):
                wt = wgtp.tile([128, 2, 1024], f8, tag=f"wqkB{t}",
                               name=f"wqkB8_{t}")
                nc.sync.dma_start(wt[:], wqkB_d[:, t])
                wqkB8.append(wt)
            for t in range(4):
                wt = wgtp.tile([128, 2, 1024], f8, tag=f"wv{t}", name=f"wv8_{t}")
                nc.sync.dma_start(wt[:], wv8_d[:, t])
                wv8.append(wt)
            for t in range(4):
                wt = wgtp.tile([128, 2, 1024], f8, tag=f"wp{t}", name=f"wp8_{t}")
                nc.sync.dma_start(wt[:], wp8_d[:, t])
                wp8.append(wt)

            SP = ["sA", "sB", "sC", "sD"]   # [128,512]  1 bank each
            # gA/gB: [128,1024] 2 banks each

            # ---------------- A1: q,k -> fp8 (+ squares), per head-pair ---
            # qk8[hp][u][p, sl, 0:512]=q / [512:1024]=k for token tile
            # 2u+sl, head-pair hp; sq8 = (q/WS)^2 etc (true squares).
            qk8 = [[], []]
            sq8 = [[], []]
            for hp in range(2):
                for u in range(8):
                    qt = qkp.tile([128, 2, 1024], f8, tag=f"qk{hp}{u}",
                                  name=f"qk8_{hp}_{u}")
                    st = qkp.tile([128, 2, 1024], f8, tag=f"sq{hp}{u}",
                                  name=f"sq8_{hp}_{u}")
                    qk8[hp].append(qt)
                    sq8[hp].append(st)

            def qk_pass(hp, wsrc):
                for u in range(8):
                    for sl in range(2):
                        s = 2 * u + sl
                        q_ps = ps1.tile([128, 512], f32, tag=SP[2 * sl],
                                        name="q_ps")
                        k_ps = ps1.tile([128, 512], f32, tag=SP[2 * sl + 1],
                                        name="k_ps")
                        hf, so = s // 8, (s % 8) * 128
                        for t in range(4):
                            nc.tensor.matmul(
                                q_ps[:], xh[t][hf][:, :, so:so + 128],
                                wsrc[t][:, :, 0:512],
                                start=(t == 0), stop=(t == 3), perf_mode=DR)
                        for t in range(4):
                            nc.tensor.matmul(
                                k_ps[:], xh[t][hf][:, :, so:so + 128],
                                wsrc[t][:, :, 512:1024],
                                start=(t == 0), stop=(t == 3), perf_mode=DR)
                        # casts on DVE, squares on ACT (parallel drains)
                        nc.vector.tensor_copy(qk8[hp][u][:, sl, 0:512], q_ps[:])
                        nc.vector.tensor_copy(qk8[hp][u][:, sl, 512:1024], k_ps[:])
                        nc.scalar.activation(sq8[hp][u][:, sl, 0:512], q_ps[:],
                                             Square, scale=1.0 / WS)
                        nc.scalar.activation(sq8[hp][u][:, sl, 512:1024], k_ps[:],
                                             Square, scale=1.0 / WS)

            def gram_ssq(hp):
                # Gram for the 2 heads of hp + channel sumsq, over all tokens
                stA = ps1.tile([128, 1024], f32, tag="gA", name=f"stA{hp}")
                ssq = ps1.tile([128, 1024], f32, tag="gB", name=f"ssq{hp}")
                for hh in range(2):
                    for m in range(2):
                        off = hh * 512 + m * 256
                        for u in range(8):
                            nc.tensor.matmul(
                                stA[:, off:off + 256],
                                qk8[hp][u][:, :, 512 + hh * 256 + m * 128:
                                           512 + hh * 256 + (m + 1) * 128],
                                qk8[hp][u][:, :, hh * 256:(hh + 1) * 256],
                                start=(u == 0), stop=(u == 7), perf_mode=DR)
                # ones stationary is M=128 wide (narrow DR ldweights fails
                # the ISA check); every psum row holds the same channel sums
                for qc in range(2):
                    for u in range(8):
                        nc.tensor.matmul(
                            ssq[:, qc * 512:(qc + 1) * 512], ones8[:],
                            sq8[hp][u][:, :, qc * 512:(qc + 1) * 512],
                            start=(u == 0), stop=(u == 7), perf_mode=DR)
                return stA, ssq

            CCN = 128 * 1024 + 1024       # Gram [128,1024] + sumsq [1,1024]
            GROUPS = [[0, 1], [2, 3], [4, 5], [6, 7]]
            cc_in = [dramp.tile([CCN], f32, name=f"cc_in{i}") for i in range(2)]
            cc_out = [dramp.tile([CCN], f32, name=f"cc_out{i}") for i in range(2)]

            strr, ssred = [], []

            def launch_cc(hp, stA, ssq):
                st_sb = wrk.tile([128, 1024], f32, tag=f"st{hp}", name=f"st_sb{hp}")
                nc.vector.tensor_copy(st_sb[:], stA[:])
                sq_sb = wrk.tile([1, 1024], f32, tag=f"sqs{hp}", name=f"sq_sb{hp}")
                nc.vector.tensor_copy(sq_sb[:], ssq[0:1, :])
                nc.sync.dma_start(
                    cc_in[hp][0:131072].rearrange("(p f) -> p f", p=128), st_sb[:])
                nc.sync.dma_start(
                    cc_in[hp][131072:CCN].rearrange("(a f) -> a f", a=1), sq_sb[:])
                nc.gpsimd.collective_compute(
                    "AllReduce", ADD, replica_groups=GROUPS,
                    ins=[cc_in[hp].opt()], outs=[cc_out[hp].opt()])
                sr = wrk.tile([128, 1024], f32, tag=f"st{hp}", name=f"str{hp}")
                nc.sync.dma_start(
                    sr[:], cc_out[hp][0:131072].rearrange("(p f) -> p f", p=128))
                strr.append(sr)
                sd = constp.tile([128, 8], f32, name=f"ssred{hp}")
                nc.sync.dma_start(
                    sd[:], cc_out[hp][131072:CCN].rearrange("(j p) -> p j", p=128))
                ssred.append(sd)

            qk_pass(0, wqkA8)
            stA0, ssq0 = gram_ssq(0)
            launch_cc(0, stA0, ssq0)

            qk_pass(1, wqkB8)
            stA1, ssq1 = gram_ssq(1)
            launch_cc(1, stA1, ssq1)

            # ---------------- phase B/C prep (decls) ----------------------
            def str_slice(h, m):
                return strr[h // 2][:, (h % 2) * 512 + m * 256:
                                    (h % 2) * 512 + (m + 1) * 256]

            rqk = []

            def emit_rqk(hp):
                # rqk[hp] cols 0-3: rq = temp/max(WS*||q||,eps); 4-7: rk
                rq = constp.tile([128, 8], f32, name=f"rqk{hp}")
                nc.scalar.activation(rq[:], ssred[hp][:], Sqrt, scale=WS * WS)
                nc.vector.tensor_scalar_max(rq[:], rq[:], 1e-9)
                nc.vector.reciprocal(rq[:], rq[:])
                nc.vector.tensor_mul(rq[:, 0:4], rq[:, 0:4],
                                     tmpv_sb[:, hp * 4:(hp + 1) * 4])
                rqk.append(rq)

            out8 = []
            v8 = []
            for h in range(H):
                ot = vop.tile([128, 2, NL], f8, tag=f"o{h}", name=f"out8_{h}")
                out8.append(ot)
                vt = vop.tile([128, 2, NL], f8, tag=f"v{h}", name=f"v8_{h}")
                v8.append(vt)
            # residual rows, resident through phase C (one DMA)
            xrbig = xsp.tile([128, 8, NL], bf, tag="xrbig", name="xrbig")
            nc.sync.dma_start(xrbig[:], xr_d[:])
            xrt = [xrbig[:, j] for j in range(8)]

            def emit_v(h):
                # v8[h][p, i, tok] = v'[h*256 + i*128 + p, tok] (fp8, xWS):
                # 4 psum chunks, dependency-free PE filler
                for dch in range(4):
                    cv = 2 * h + dch // 2
                    half, i = dch % 2, cv % 2
                    vp = ps1.tile([128, 1024], f32,
                                  tag=("gA" if dch % 2 == 0 else "gB"),
                                  name="vp")
                    for c in range(2):
                        for t in range(4):
                            nc.tensor.matmul(
                                vp[:, c * 512:(c + 1) * 512],
                                wv8[t][:, :, cv * 128:(cv + 1) * 128],
                                xh[t][half][:, :, c * 512:(c + 1) * 512],
                                start=(t == 0), stop=(t == 3), perf_mode=DR)
                    nc.vector.tensor_copy(
                        v8[h][:, i, half * 1024:(half + 1) * 1024], vp[:])

            def attn_s1a(h):
                # rows d scaled by rk[d] -> bf16, then transpose to S[c,d]
                hp, hh = h // 2, h % 2
                sth = wrk.tile([128, 512], bf, tag="sth", bufs=2, name="sth")
                for m in range(2):
                    nc.vector.tensor_scalar_mul(
                        sth[:, m * 256:(m + 1) * 256], str_slice(h, m),
                        rqk[hp][:, 4 + 2 * hh + m: 5 + 2 * hh + m])
                spm = ps1.tile([128, 512], bf, tag="sA", name="spm")
                for mc in range(2):
                    for md in range(2):
                        nc.tensor.transpose(
                            spm[:, mc * 256 + md * 128: mc * 256 + (md + 1) * 128],
                            sth[:, md * 256 + mc * 128: md * 256 + (mc + 1) * 128],
                            identb[:])
                return spm

            def attn_s1b(h, spm):
                # softmax over d. Scores are bounded (|s| <= temp since q,k
                # are unit vectors): skip max-subtraction, rq folds into the
                # Exp scale. Then attn^T via bf16 PE transposes.
                hp, hh = h // 2, h % 2
                rowsum = wrk.tile([128, 2], f32, tag="rowsum", bufs=2,
                                  name="rowsum")
                recip = wrk.tile([128, 2], f32, tag="recip", bufs=2,
                                 name="recip")
                esb = wrk.tile([128, 512], bf, tag="esb", bufs=2, name="esb")
                for mc in range(2):
                    nc.scalar.activation(esb[:, mc * 256:(mc + 1) * 256],
                                         spm[:, mc * 256:(mc + 1) * 256],
                                         Exp,
                                         scale=rqk[hp][:, 2 * hh + mc:
                                                       1 + 2 * hh + mc],
                                         accum_out=rowsum[:, mc:mc + 1])
                nc.vector.reciprocal(recip[:], rowsum[:])
                atp = ps1.tile([128, 512], bf, tag="sB", name="atp")
                for md in range(2):
                    for mc in range(2):
                        nc.tensor.transpose(
                            atp[:, md * 256 + mc * 128: md * 256 + (mc + 1) * 128],
                            esb[:, mc * 256 + md * 128: mc * 256 + (md + 1) * 128],
                            identb[:])
                atn8 = wrk.tile([128, 2, 256], f8, tag="atn8", bufs=2,
                                name="atn8")
                nc.scalar.activation(atn8[:, 0, :], atp[:, 0:256], Ident)
                nc.scalar.activation(atn8[:, 1, :], atp[:, 256:512], Ident)
                return atn8, recip

            # ---------------- phase B+C with v-pass as PE filler ----------
            # Head h needs only collective #hp and v8[h]; v8[h+1]'s chunks
            # are always-ready PE work emitted inside head h's softmax
            # latency window. emit_rqk placed so its collective-gated ops
            # never block casts queued behind them.
            emit_v(0)
            emit_rqk(0)
            for h in range(H):
                spm = attn_s1a(h)
                if h + 1 < H:
                    emit_v(h + 1)
                atn8, recip = attn_s1b(h, spm)
                if h == 1:
                    emit_rqk(1)
                # out' = attn^T(unnorm) @ v', row-scaled by 1/rowsum
                for nfh in range(2):
                    opc = ps1.tile([128, 1024], f32, tag="gA", name="opc")
                    opd = ps1.tile([128, 1024], f32, tag="gB", name="opd")
                    for mc, op in ((0, opc), (1, opd)):
                        for nf2 in range(2):
                            nc.tensor.matmul(
                                op[:, nf2 * 512:(nf2 + 1) * 512],
                                atn8[:, :, mc * 128:(mc + 1) * 128],
                                v8[h][:, :, nfh * 1024 + nf2 * 512:
                                      nfh * 1024 + (nf2 + 1) * 512],
                                start=True, stop=True, perf_mode=DR)
                    for mc, op in ((0, opc), (1, opd)):
                        nc.vector.tensor_scalar_mul(
                            out8[h][:, mc, nfh * 1024:(nfh + 1) * 1024],
                            op[:], recip[:, mc:mc + 1])

                # ---- projection for token quarter q=h (sC/sD) ----
                for j in range(8):
                    pq = ps1.tile([128, 512], f32, tag=SP[2 + j % 2], name="pq")
                    for u in range(4):
                        nc.tensor.matmul(
                            pq[:], wp8[u][:, :, j * 128:(j + 1) * 128],
                            out8[h][:, :, u * 512:(u + 1) * 512],
                            start=(u == 0), stop=(u == 3), perf_mode=DR)
                    yq = wrk.tile([128, 512], bf, tag=f"yq{j % 2}", bufs=2,
                                  name=f"yq_{h}_{j}")
                    nc.scalar.activation(yq[:], pq[:], Ident,
                                         bias=bias_sb[:, j:j + 1],
                                         scale=1.0 / (WS * WS))
                    eng = nc.vector if j % 2 == 0 else nc.gpsimd
                    eng.tensor_add(yq[:], yq[:],
                                   xrt[j][:, h * 512:(h + 1) * 512])
                    nc.sync.dma_start(
                        yT_d[j * 128:(j + 1) * 128, h * 512:(h + 1) * 512],
                        yq[:])

    nc.compile()
    return nc


def _get_nc():
    if "nc" not in _CACHE:
        _CACHE["nc"] = _build()
    return _CACHE["nc"]


def _dr_pack(a):
    """[C, F] channel-major -> DoubleRow [128, C/256, 2, F]: partition p,
    tile t, group i holds channel 256t+128i+p (single-DMA layout)."""
    Cc, F = a.shape
    return np.ascontiguousarray(
        a.reshape(Cc // 256, 2, 128, F).transpose(2, 0, 1, 3))


def _out_rows(half):
    # torch transpose+reshape scramble: this core's y rows
    return np.concatenate(
        [h * 1024 + half * 512 + np.arange(512) for h in range(H)])


def _make_in_maps(x, Wqkv, Wproj, bproj, temperature):
    x = np.ascontiguousarray(np.asarray(x, dtype=np.float32))
    Wqkv = np.asarray(Wqkv, dtype=np.float32)
    Wproj = np.asarray(Wproj, dtype=np.float32)
    bproj = np.asarray(bproj, dtype=np.float32).reshape(C)
    temp = np.asarray(temperature, dtype=np.float32).reshape(H)

    WqkvT = np.ascontiguousarray(Wqkv.T) * WS     # [C, 3C], prescaled
    wqkA = _dr_pack(np.concatenate(
        [WqkvT[:, 0:512], WqkvT[:, C:C + 512]], axis=1)).astype(F8)
    wqkB = _dr_pack(np.concatenate(
        [WqkvT[:, 512:C], WqkvT[:, C + 512:2 * C]], axis=1)).astype(F8)
    wv8 = _dr_pack(WqkvT[:, 2 * C:]).astype(F8)
    wp8 = _dr_pack(np.ascontiguousarray(Wproj.T) * WS).astype(F8)
    bias2d = np.ascontiguousarray(bproj.reshape(8, 128).T)
    tmpv2d = np.ascontiguousarray(np.repeat(temp, HD).reshape(8, 128).T)

    # token permutation: position u*512+n holds token 4n+u. Gram, norms
    # and attention are token-order invariant; the projection's stride-4
    # token gather becomes a contiguous slice.
    perm = np.concatenate([np.arange(u, NL, 4) for u in range(4)])
    in_maps = []
    for core in range(NCORES):
        b, half = core // 2, core % 2
        xT = np.ascontiguousarray(x[b, half * NL:(half + 1) * NL, :].T)
        x8 = _dr_pack(xT[:, perm]).astype(F8)
        xr = np.ascontiguousarray(
            x[b, _out_rows(half), :].T.reshape(8, 128, NL).transpose(
                1, 0, 2)).astype(BF16)
        in_maps.append(dict(x8=x8, xr=xr, wqkA=wqkA, wqkB=wqkB, wv8=wv8,
                            wp8=wp8, bias=bias2d, tmpv=tmpv2d))
    return in_maps


def _run(in_maps, trace=False, **kw):
    from concourse.bass_utils import run_bass_kernel_spmd

    nc = _get_nc()
    return run_bass_kernel_spmd(nc, in_maps, core_ids=list(range(NCORES)),
                                trace=trace, **kw)


def kernel(x, Wqkv, Wproj, bproj, temperature):
    res = _run(_make_in_maps(x, Wqkv, Wproj, bproj, temperature))
    y = np.empty((B, N, C), dtype=np.float32)
    for core in range(NCORES):
        b, half = core // 2, core % 2
        y[b, _out_rows(half), :] = res.results[core]["yT"].astype(np.float32).T
    return y


# revision 33
# speedup vs baseline: 1.0070x; 1.0070x over previous
"""Efficient Channel Attention kernel for 8 Trainium2 NeuronCores.

Problem (B=4, N=4096, C=1024, H=4, HD=256):
    qkv = x @ Wqkv.T                 -> q,k,v per head
    q,k l2-normalized over N; scores = (q*temp) @ k.T   [HD, HD] per (b,h)
    attn = softmax(scores, -1); out = attn @ v; y = out @ Wproj.T + bproj + x

Sharding: core = (batch b, token-half). All channel contractions are local;
the only cross-core coupling is the token(N)-contracted quantities: the
Grams k^T q and the q/k squared norms, AllReduce'd within the core pair
sharing a batch. The reduction is split by HEAD-PAIR: q,k for heads 0,1 are
computed first, their Gram+norms AllReduce (#1, 528KB) flies while heads
2,3 are computed (#2 likewise overlaps the v pass), so neither collective's
~25us latency is exposed.

All large GEMMs run in fp8(e4m3) with MatmulPerfMode.DoubleRow: operands
laid out [K=128, 2, free] (two 128-row contraction groups per instruction,
0.5 cycles/row = 2x bf16/f32r PE rate). Weights are pre-scaled by WS=32 on
the host so W entries sit in fp8's normal range (q/k/v reach ~7.7 abs, so
32x keeps everything under fp8 e4m3's 448 max). The power-of-two scales
cancel in l2-normalization and are folded into the norm reciprocals and the
final projection bias-activation (scale=1/WS^2).

PSUM discipline (8 banks): sA-sD [128,512] (1 bank each) + gA,gB
[128,1024] (2 banks each). A1 q/k tiles double-buffer across sA-sD while
gA/gB hold Gram/norm accumulators; in phase B/C, attention h uses
sA/sB/gA/gB while projection h rides sC/sD under attention h+1.
"""

import numpy as np
import ml_dtypes

B, N, C, H = 4, 4096, 1024, 4
HD = C // H          # 256
NCORES = 8
NL = N // 2          # 2048 tokens per core
WS = 32.0            # host-side weight prescale
F8 = ml_dtypes.float8_e4m3
BF16 = ml_dtypes.bfloat16

_CACHE = {}


def _build():
    import concourse.mybir as mybir
    import concourse.tile as tile
    from concourse import bacc
    from concourse.masks import make_identity

    f32 = mybir.dt.float32
    bf = mybir.dt.bfloat16
    f8 = mybir.dt.float8e4
    DR = mybir.MatmulPerfMode.DoubleRow
    AX = mybir.AxisListType.X
    ADD = mybir.AluOpType.add
    Exp = mybir.ActivationFunctionType.Exp
    Ident = mybir.ActivationFunctionType.Identity
    Square = mybir.ActivationFunctionType.Square
    Sqrt = mybir.ActivationFunctionType.Sqrt

    nc = bacc.Bacc("TRN2", target_bir_lowering=False, debug=False,
                   num_devices=NCORES)

    x8_d = nc.dram_tensor("x8", [128, 4, 2, NL], f8, kind="ExternalInput").ap()
    # wqkA = (q cols 0:512 | k cols 0:512) i.e. heads 0,1; wqkB = heads 2,3
    wqkA_d = nc.dram_tensor("wqkA", [128, 4, 2, 1024], f8, kind="ExternalInput").ap()
    wqkB_d = nc.dram_tensor("wqkB", [128, 4, 2, 1024], f8, kind="ExternalInput").ap()
    wv8_d = nc.dram_tensor("wv8", [128, 4, 2, 1024], f8, kind="ExternalInput").ap()
    wp8_d = nc.dram_tensor("wp8", [128, 4, 2, 1024], f8, kind="ExternalInput").ap()
    xr_d = nc.dram_tensor("xr", [128, 8, NL], bf, kind="ExternalInput").ap()
    bias_d = nc.dram_tensor("bias", [128, 8], f32, kind="ExternalInput").ap()
    tmpv_d = nc.dram_tensor("tmpv", [128, 8], f32, kind="ExternalInput").ap()
    yT_d = nc.dram_tensor("yT", [C, NL], bf, kind="ExternalOutput").ap()

    with tile.TileContext(nc) as tc:
        with (
            tc.tile_pool(name="const", bufs=1) as constp,
            tc.tile_pool(name="wgt", bufs=1) as wgtp,
            tc.tile_pool(name="xs", bufs=1) as xsp,
            tc.tile_pool(name="qk", bufs=1) as qkp,
            tc.tile_pool(name="vo", bufs=1) as vop,
            tc.tile_pool(name="wrk", bufs=1) as wrk,
            tc.tile_pool(name="ps1", bufs=1, space="PSUM") as ps1,
            tc.tile_pool(name="dram", bufs=1, space="DRAM") as dramp,
        ):
            # ---------------- constants ----------------
            ident = constp.tile([128, 128], f32, name="ident")
            make_identity(nc, ident[:])
            identb = constp.tile([128, 128], bf, name="identb")
            nc.gpsimd.tensor_copy(identb[:], ident[:])
            bias_sb = constp.tile([128, 8], f32, name="bias_sb")
            nc.sync.dma_start(bias_sb[:], bias_d[:])
            tmpv_sb = constp.tile([128, 8], f32, name="tmpv_sb")
            nc.sync.dma_start(tmpv_sb[:], tmpv_d[:])
            ones8 = constp.tile([128, 2, 128], f8, name="ones8")
            nc.vector.memset(ones8[:], 1.0)

            # ---------------- bulk input DMA (all SBUF-resident) ---------
            # many mid-size DMAs stream on parallel DMA engines,
            # first-needed-first: x token-half 0 + wqkA gate the first
            # matmul (~1.5MB), everything else arrives behind it
            # issue queues round-robin so 600ns/issue doesn't serialize
            xh = [[None, None] for _ in range(4)]
            for t in range(4):
                xt = xsp.tile([128, 2, 1024], f8, tag=f"x{t}0", name=f"x8_{t}0")
                nc.sync.dma_start(xt[:], x8_d[:, t, :, 0:1024])
                xh[t][0] = xt
            wqkA8 = []
            for t in range(4):
                wt = wgtp.tile([128, 2, 1024], f8, tag=f"wqkA{t}",
                               name=f"wqkA8_{t}")
                nc.sync.dma_start(wt[:], wqkA_d[:, t])
                wqkA8.append(wt)
            for t in range(4):
                xt = xsp.tile([128, 2, 1024], f8, tag=f"x{t}1", name=f"x8_{t}1")
                nc.sync.dma_start(xt[:], x8_d[:, t, :, 1024:2048])
                xh[t][1] = xt
            wqkB8, wv8, wp8 = [], [], []
            for t in range(4):
                wt = wgtp.tile([128, 2, 1024], f8, tag=f"wqkB{t}",
                               name=f"wqkB8_{t}")
                nc.sync.dma_start(wt[:], wqkB_d[:, t])
                wqkB8.append(wt)
            for t in range(4):
                wt = wgtp.tile([128, 2, 1024], f8, tag=f"wv{t}", name=f"wv8_{t}")
                nc.sync.dma_start(wt[:], wv8_d[:, t])
                wv8.append(wt)
            for t in range(4):
                wt = wgtp.tile([128, 2, 1024], f8, tag=f"wp{t}", name=f"wp8_{t}")
                nc.sync.dma_start(wt[:], wp8_d[:, t])
                wp8.append(wt)

            SP = ["sA", "sB", "sC", "sD"]   # [128,512]  1 bank each
            # gA/gB: [128,1024] 2 banks each

            # ---------------- A1: q,k -> fp8 (+ squares), per head-pair ---
            # qk8[hp][u][p, sl, 0:512]=q / [512:1024]=k for token tile
            # 2u+sl, head-pair hp; sq8 = (q/WS)^2 etc (true squares).
            qk8 = [[], []]
            sq8 = [[], []]
            for hp in range(2):
                for u in range(8):
                    qt = qkp.tile([128, 2, 1024], f8, tag=f"qk{hp}{u}",
                                  name=f"qk8_{hp}_{u}")
                    st = qkp.tile([128, 2, 1024], f8, tag=f"sq{hp}{u}",
                                  name=f"sq8_{hp}_{u}")
                    qk8[hp].append(qt)
                    sq8[hp].append(st)

            def qk_pass(hp, wsrc):
                for u in range(8):
                    for sl in range(2):
                        s = 2 * u + sl
                        q_ps = ps1.tile([128, 512], f32, tag=SP[2 * sl],
                                        name="q_ps")
                        k_ps = ps1.tile([128, 512], f32, tag=SP[2 * sl + 1],
                                        name="k_ps")
                        hf, so = s // 8, (s % 8) * 128
                        for t in range(4):
                            nc.tensor.matmul(
                                q_ps[:], xh[t][hf][:, :, so:so + 128],
                                wsrc[t][:, :, 0:512],
                                start=(t == 0), stop=(t == 3), perf_mode=DR)
                        for t in range(4):
                            nc.tensor.matmul(
                                k_ps[:], xh[t][hf][:, :, so:so + 128],
                                wsrc[t][:, :, 512:1024],
                                start=(t == 0), stop=(t == 3), perf_mode=DR)
                        # casts on DVE, squares on ACT (parallel drains)
                        nc.vector.tensor_copy(qk8[hp][u][:, sl, 0:512], q_ps[:])
                        nc.vector.tensor_copy(qk8[hp][u][:, sl, 512:1024], k_ps[:])
                        nc.scalar.activation(sq8[hp][u][:, sl, 0:512], q_ps[:],
                                             Square, scale=1.0 / WS)
                        nc.scalar.activation(sq8[hp][u][:, sl, 512:1024], k_ps[:],
                                             Square, scale=1.0 / WS)

            def gram_ssq(hp):
                # Gram for the 2 heads of hp + channel sumsq, over all tokens
                stA = ps1.tile([128, 1024], f32, tag="gA", name=f"stA{hp}")
                ssq = ps1.tile([128, 1024], f32, tag="gB", name=f"ssq{hp}")
                for hh in range(2):
                    for m in range(2):
                        off = hh * 512 + m * 256
                        for u in range(8):
                            nc.tensor.matmul(
                                stA[:, off:off + 256],
                                qk8[hp][u][:, :, 512 + hh * 256 + m * 128:
                                           512 + hh * 256 + (m + 1) * 128],
                                qk8[hp][u][:, :, hh * 256:(hh + 1) * 256],
                                start=(u == 0), stop=(u == 7), perf_mode=DR)
                # ones stationary is M=128 wide (narrow DR ldweights fails
                # the ISA check); every psum row holds the same channel sums
                for qc in range(2):
                    for u in range(8):
                        nc.tensor.matmul(
                            ssq[:, qc * 512:(qc + 1) * 512], ones8[:],
                            sq8[hp][u][:, :, qc * 512:(qc + 1) * 512],
                            start=(u == 0), stop=(u == 7), perf_mode=DR)
                return stA, ssq

            CCN = 128 * 1024 + 1024       # Gram [128,1024] + sumsq [1,1024]
            GROUPS = [[0, 1], [2, 3], [4, 5], [6, 7]]
            cc_in = [dramp.tile([CCN], f32, name=f"cc_in{i}") for i in range(2)]
            cc_out = [dramp.tile([CCN], f32, name=f"cc_out{i}") for i in range(2)]

            strr, ssred = [], []

            def launch_cc(hp, stA, ssq):
                st_sb = wrk.tile([128, 1024], f32, tag=f"st{hp}", name=f"st_sb{hp}")
                nc.vector.tensor_copy(st_sb[:], stA[:])
                sq_sb = wrk.tile([1, 1024], f32, tag=f"sqs{hp}", name=f"sq_sb{hp}")
                nc.vector.tensor_copy(sq_sb[:], ssq[0:1, :])
                nc.sync.dma_start(
                    cc_in[hp][0:131072].rearrange("(p f) -> p f", p=128), st_sb[:])
                nc.sync.dma_start(
                    cc_in[hp][131072:CCN].rearrange("(a f) -> a f", a=1), sq_sb[:])
                nc.gpsimd.collective_compute(
                    "AllReduce", ADD, replica_groups=GROUPS,
                    ins=[cc_in[hp].opt()], outs=[cc_out[hp].opt()])
                sr = wrk.tile([128, 1024], f32, tag=f"st{hp}", name=f"str{hp}")
                nc.sync.dma_start(
                    sr[:], cc_out[hp][0:131072].rearrange("(p f) -> p f", p=128))
                strr.append(sr)
                sd = constp.tile([128, 8], f32, name=f"ssred{hp}")
                nc.sync.dma_start(
                    sd[:], cc_out[hp][131072:CCN].rearrange("(j p) -> p j", p=128))
                ssred.append(sd)

            qk_pass(0, wqkA8)
            stA0, ssq0 = gram_ssq(0)
            launch_cc(0, stA0, ssq0)

            qk_pass(1, wqkB8)
            stA1, ssq1 = gram_ssq(1)
            launch_cc(1, stA1, ssq1)

            # ---------------- A2: v (overlaps collective #2) --------------
            # v8[h][p, i, tok] = v'[h*256 + i*128 + p, tok]  (fp8, xWS)
            v8 = []
            for h in range(H):
                vt = vop.tile([128, 2, NL], f8, tag=f"v{h}", name=f"v8_{h}")
                v8.append(vt)
            for ch in range(16):            # [128,1024] psum chunks
                cv, half = ch // 2, ch % 2
                h, i = cv // 2, cv % 2
                vp = ps1.tile([128, 1024], f32, tag=("gA" if ch % 2 == 0 else "gB"),
                              name="vp")
                for c in range(2):
                    for t in range(4):
                        nc.tensor.matmul(
                            vp[:, c * 512:(c + 1) * 512],
                            wv8[t][:, :, cv * 128:(cv + 1) * 128],
                            xh[t][half][:, :, c * 512:(c + 1) * 512],
                            start=(t == 0), stop=(t == 3), perf_mode=DR)
                nc.vector.tensor_copy(
                    v8[h][:, i, half * 1024:(half + 1) * 1024], vp[:])

            # ---------------- phase B/C prep ------------------------------
            # rqk[hp] cols 0-3: rq = temp/max(WS*||q||,eps) for the 512
            # q-channels of hp; cols 4-7: rk = 1/(WS*||k||)
            rqk = []
            for hp in range(2):
                rq = constp.tile([128, 8], f32, name=f"rqk{hp}")
                nc.scalar.activation(rq[:], ssred[hp][:], Sqrt, scale=WS * WS)
                nc.vector.tensor_scalar_max(rq[:], rq[:], 1e-9)
                nc.vector.reciprocal(rq[:], rq[:])
                nc.vector.tensor_mul(rq[:, 0:4], rq[:, 0:4],
                                     tmpv_sb[:, hp * 4:(hp + 1) * 4])
                rqk.append(rq)

            def str_slice(h, m):
                return strr[h // 2][:, (h % 2) * 512 + m * 256:
                                    (h % 2) * 512 + (m + 1) * 256]

            out8 = []
            for h in range(H):
                ot = vop.tile([128, 2, NL], f8, tag=f"o{h}", name=f"out8_{h}")
                out8.append(ot)
            # residual rows, resident through phase C (one DMA)
            xrbig = xsp.tile([128, 8, NL], bf, tag="xrbig", name="xrbig")
            nc.sync.dma_start(xrbig[:], xr_d[:])
            xrt = [xrbig[:, j] for j in range(8)]

            # ---------------- phase B+C, pipelined per head ---------------
            # attention(h) on sA/sB/gA/gB; projection(h) on sC/sD so it
            # overlaps attention(h+1)
            for h in range(H):
                hp, hh = h // 2, h % 2
                # rows d scaled by rk[d] -> bf16
                sth = wrk.tile([128, 512], bf, tag="sth", bufs=2, name="sth")
                for m in range(2):
                    nc.vector.tensor_scalar_mul(
                        sth[:, m * 256:(m + 1) * 256], str_slice(h, m),
                        rqk[hp][:, 4 + 2 * hh + m: 5 + 2 * hh + m])
                spm = ps1.tile([128, 512], bf, tag="sA", name="spm")
                for mc in range(2):
                    for md in range(2):
                        nc.tensor.transpose(
                            spm[:, mc * 256 + md * 128: mc * 256 + (md + 1) * 128],
                            sth[:, md * 256 + mc * 128: md * 256 + (mc + 1) * 128],
                            identb[:])
                # softmax over d (free axis). Scores are bounded
                # (|s| <= temp since q,k are unit vectors), so skip the
                # max-subtraction; rq folds into the Exp's per-row scale.
                rowsum = wrk.tile([128, 2], f32, tag="rowsum", bufs=2, name="rowsum")
                recip = wrk.tile([128, 2], f32, tag="recip", bufs=2, name="recip")
                esb = wrk.tile([128, 512], bf, tag="esb", bufs=2, name="esb")
                for mc in range(2):
                    nc.scalar.activation(esb[:, mc * 256:(mc + 1) * 256],
                                         spm[:, mc * 256:(mc + 1) * 256],
                                         Exp,
                                         scale=rqk[hp][:, 2 * hh + mc:
                                                       1 + 2 * hh + mc],
                                         accum_out=rowsum[:, mc:mc + 1])
                nc.vector.reciprocal(recip[:], rowsum[:])
                # attn^T [d, (md, c)] via bf16 PE transposes (fp8 transpose
                # needs stride-2 psum writes per the walrus verifier)
                atp = ps1.tile([128, 512], bf, tag="sB", name="atp")
                for md in range(2):
                    for mc in range(2):
                        nc.tensor.transpose(
                            atp[:, md * 256 + mc * 128: md * 256 + (mc + 1) * 128],
                            esb[:, mc * 256 + md * 128: mc * 256 + (md + 1) * 128],
                            identb[:])
                atn8 = wrk.tile([128, 2, 256], f8, tag="atn8", bufs=2, name="atn8")
                nc.scalar.activation(atn8[:, 0, :], atp[:, 0:256], Ident)
                nc.scalar.activation(atn8[:, 1, :], atp[:, 256:512], Ident)
                # out' = attn^T(unnorm) @ v', row-scaled by 1/rowsum
                for nfh in range(2):
                    opc = ps1.tile([128, 1024], f32, tag="gA", name="opc")
                    opd = ps1.tile([128, 1024], f32, tag="gB", name="opd")
                    for mc, op in ((0, opc), (1, opd)):
                        for nf2 in range(2):
                            nc.tensor.matmul(
                                op[:, nf2 * 512:(nf2 + 1) * 512],
                                atn8[:, :, mc * 128:(mc + 1) * 128],
                                v8[h][:, :, nfh * 1024 + nf2 * 512:
                                      nfh * 1024 + (nf2 + 1) * 512],
                                start=True, stop=True, perf_mode=DR)
                    for mc, op in ((0, opc), (1, opd)):
                        nc.vector.tensor_scalar_mul(
                            out8[h][:, mc, nfh * 1024:(nfh + 1) * 1024],
                            op[:], recip[:, mc:mc + 1])

                # ---- projection for token quarter q=h (sC/sD) ----
                for j in range(8):
                    pq = ps1.tile([128, 512], f32, tag=SP[2 + j % 2], name="pq")
                    for u in range(4):
                        nc.tensor.matmul(
                            pq[:], wp8[u][:, :, j * 128:(j + 1) * 128],
                            out8[h][:, :, u * 512:(u + 1) * 512],
                            start=(u == 0), stop=(u == 3), perf_mode=DR)
                    yq = wrk.tile([128, 512], bf, tag=f"yq{j % 2}", bufs=2,
                                  name=f"yq_{h}_{j}")
                    nc.scalar.activation(yq[:], pq[:], Ident,
                                         bias=bias_sb[:, j:j + 1],
                                         scale=1.0 / (WS * WS))
                    eng = nc.vector if j % 2 == 0 else nc.gpsimd
                    eng.tensor_add(yq[:], yq[:],
                                   xrt[j][:, h * 512:(h + 1) * 512])
                    nc.sync.dma_start(
                        yT_d[j * 128:(j + 1) * 128, h * 512:(h + 1) * 512],
                        yq[:])

    nc.compile()
    return nc


def _get_nc():
    if "nc" not in _CACHE:
        _CACHE["nc"] = _build()
    return _CACHE["nc"]


def _dr_pack(a):
    """[C, F] channel-major -> DoubleRow [128, C/256, 2, F]: partition p,
    tile t, group i holds channel 256t+128i+p (single-DMA layout)."""
    Cc, F = a.shape
    return np.ascontiguousarray(
        a.reshape(Cc // 256, 2, 128, F).transpose(2, 0, 1, 3))


def _out_rows(half):
    # torch transpose+reshape scramble: this core's y rows
    return np.concatenate(
        [h * 1024 + half * 512 + np.arange(512) for h in range(H)])


def _make_in_maps(x, Wqkv, Wproj, bproj, temperature):
    x = np.ascontiguousarray(np.asarray(x, dtype=np.float32))
    Wqkv = np.asarray(Wqkv, dtype=np.float32)
    Wproj = np.asarray(Wproj, dtype=np.float32)
    bproj = np.asarray(bproj, dtype=np.float32).reshape(C)
    temp = np.asarray(temperature, dtype=np.float32).reshape(H)

    WqkvT = np.ascontiguousarray(Wqkv.T) * WS     # [C, 3C], prescaled
    wqkA = _dr_pack(np.concatenate(
        [WqkvT[:, 0:512], WqkvT[:, C:C + 512]], axis=1)).astype(F8)
    wqkB = _dr_pack(np.concatenate(
        [WqkvT[:, 512:C], WqkvT[:, C + 512:2 * C]], axis=1)).astype(F8)
    wv8 = _dr_pack(WqkvT[:, 2 * C:]).astype(F8)
    wp8 = _dr_pack(np.ascontiguousarray(Wproj.T) * WS).astype(F8)
    bias2d = np.ascontiguousarray(bproj.reshape(8, 128).T)
    tmpv2d = np.ascontiguousarray(np.repeat(temp, HD).reshape(8, 128).T)

    # token permutation: position u*512+n holds token 4n+u. Gram, norms
    # and attention are token-order invariant; the projection's stride-4
    # token gather becomes a contiguous slice.
    perm = np.concatenate([np.arange(u, NL, 4) for u in range(4)])
    in_maps = []
    for core in range(NCORES):
        b, half = core // 2, core % 2
        xT = np.ascontiguousarray(x[b, half * NL:(half + 1) * NL, :].T)
        x8 = _dr_pack(xT[:, perm]).astype(F8)
        xr = np.ascontiguousarray(
            x[b, _out_rows(half), :].T.reshape(8, 128, NL).transpose(
                1, 0, 2)).astype(BF16)
        in_maps.append(dict(x8=x8, xr=xr, wqkA=wqkA, wqkB=wqkB, wv8=wv8,
                            wp8=wp8, bias=bias2d, tmpv=tmpv2d))
    return in_maps


def _run(in_maps, trace=False, **kw):
    from concourse.bass_utils import run_bass_kernel_spmd

    nc = _get_nc()
    return run_bass_kernel_spmd(nc, in_maps, core_ids=list(range(NCORES)),
                                trace=trace, **kw)


def kernel(x, Wqkv, Wproj, bproj, temperature):
    res = _run(_make_in_maps(x, Wqkv, Wproj, bproj, temperature))
    y = np.empty((B, N, C), dtype=np.float32)
    for core in range(NCORES):
        b, half = core // 2, core % 2
        y[b, _out_rows(half), :] = res.results[core]["yT"].astype(np.float32).T
    return y
